# revision 17
# baseline (speedup 1.0000x reference)
"""Bass/Trainium2 kernel for GruAttCosMeanNet (nn_GruAttCosMeanNet_39591008535146).

Data-parallel over batch: 8 cores x 2 batch rows each.

v2: latency-optimized GRU recurrences.
  - gates packed [r | z | zbar | n] (zbar = sigmoid of negated z-arg), so the
    update h' = z*h + zbar*n needs no (1-z) op.
  - split state h = m1 + m2 (m1 = z*h_prev off the critical path, m2 = zbar*n
    on it); the recurrence matmuls accumulate Wh*m1 + Wh*m2.
  - input projections computed inside the loop as PSUM-accumulated matmuls
    (x carries a ones-row so Wi biases fold in; bh_n enters via a 1-row
    matmul so it stays inside the r-gate product).
  - off-critical elementwise (m1, h-sum, enc stores, mean accumulators) on
    GpSimd; critical path: MM -> sig(r,z,zbar) -> r*hpn -> +xpn -> tanh ->
    zbar*n -> MM.
  - bwd direction reads the same fwd-ordered x via index T-1-t; opt cols
    (len 64) are stored twice (offset 0 and 64) so one index works for both.
"""
import sys
sys.path.insert(0, "/opt/trn_rl_repo")
import numpy as np
import ml_dtypes

import concourse.bass as bass
import concourse.mybir as mybir
import concourse.tile as tile
from concourse import bacc, bass_utils
from concourse.masks import make_identity

BF16 = mybir.dt.bfloat16
F32 = mybir.dt.float32
AF = mybir.ActivationFunctionType
ALU = mybir.AluOpType

B, LC, LO, NOPT, E, H = 16, 128, 64, 5, 300, 256
NCORES = 8
BL = B // NCORES          # 2 batch rows per core
NI = BL * NOPT            # 10 (b,opt) pairs per core
NBM = BL + NI             # 12 cols in main GRU (2 ctx + 10 opt)
NBA = 2 * NI              # 20 cols in att GRU (10 actx + 10 aopt)
H3 = 3 * H                # 768
bf = ml_dtypes.bfloat16

_CACHE = {}


def _build():
    nc = bacc.Bacc("TRN2", target_bir_lowering=False, debug=False,
                   num_devices=NCORES)

    d = {}
    # unified main x: [3k, 128, 128t*12cols] (ctx cols 0:2, opt cols 2:12;
    # opt rows duplicated at t 0:64 and 64:128)
    d["xu"] = nc.dram_tensor("xu", [3, 128, LC * NBM], BF16, kind="ExternalInput")
    # gate-packed weights: cols [r(256)|z(256)|zbar(256)|n(256)]
    d["wi8m"] = nc.dram_tensor("wi8m", [2, 3, 128, 1024], BF16, kind="ExternalInput")
    d["wh8m"] = nc.dram_tensor("wh8m", [2, 2, 128, 1024], BF16, kind="ExternalInput")
    d["wi8a"] = nc.dram_tensor("wi8a", [2, 2, 128, 1024], BF16, kind="ExternalInput")
    d["wh8a"] = nc.dram_tensor("wh8a", [2, 2, 128, 1024], BF16, kind="ExternalInput")
    d["bhnm"] = nc.dram_tensor("bhnm", [2, 256], BF16, kind="ExternalInput")  # [dir, n-cols]
    d["bhna"] = nc.dram_tensor("bhna", [2, 256], BF16, kind="ExternalInput")
    d["batt"] = nc.dram_tensor("batt", [2, 1024], BF16, kind="ExternalInput")  # att xp biases
    d["wk"] = nc.dram_tensor("wk", [4, 128, H], BF16, kind="ExternalInput")
    d["wq"] = nc.dram_tensor("wq", [4, 128, H], BF16, kind="ExternalInput")
    d["v"] = nc.dram_tensor("v", [128, 2], F32, kind="ExternalInput")
    d["out"] = nc.dram_tensor("out", [1, NI], F32, kind="ExternalOutput")

    with tile.TileContext(nc) as tc:
        _body(nc, tc, d)
    nc.compile()
    return nc


def _gru_loop(nc, tc, pools, wh8, wi8, xsrc, nxk, bias_rows, bhn_row, onesrow,
              nb, store):
    """Shared bidirectional GRU loop, 128 steps.

    wh8: [128, 2dir, 2k, 1024] stationary (gate-packed).
    wi8: [128, 2dir, nxk, 1024] or per-dir-shared [128, nxk, 1024] (att).
    xsrc(d, k, t): moving AP [128, nb] for x input at fwd-index t.
    bias_rows: None or tile [1, 2dir, 1024] (att xp biases).
    bhn_row: [1, 2dir, 256] (bh_n, hp-side).
    store(d, t, g): store hidden state g [128, 2, nb] at step t.
    """
    psg, sbp = pools
    m1 = [None, None]
    m2 = [None, None]
    gt = [None, None]
    rzs = [None, None]
    narg = [None, None]
    sst = [None, None]
    for dd in range(2):
        m1[dd] = sbp.tile([128, 2, nb], BF16, tag=f"m1_{nb}_{dd}")
        m2[dd] = sbp.tile([128, 2, nb], BF16, tag=f"m2_{nb}_{dd}")
        gt[dd] = sbp.tile([128, 2, nb], BF16, tag=f"g_{nb}_{dd}")
        rzs[dd] = sbp.tile([128, 6, nb], F32, tag=f"rz_{nb}_{dd}")
        narg[dd] = sbp.tile([128, 2, nb], F32, tag=f"na_{nb}_{dd}")
        sst[dd] = sbp.tile([128, 2, nb], F32, tag=f"ss_{nb}_{dd}")
        nc.vector.memset(m1[dd][:], 0.0)
        nc.vector.memset(m2[dd][:], 0.0)
        nc.vector.memset(gt[dd][:], 0.0)

    def emit_xp(dd, t2, P):
        # x-side MMs + bias rows: no data deps, dispatch ahead
        for jg in range(6):
            for k in range(nxk):
                nc.tensor.matmul(P[:, jg], wi8[:, dd, k, jg * 128:(jg + 1) * 128],
                                 xsrc(dd, k, t2), start=(k == 0), stop=False)
            if bias_rows is not None:
                nc.tensor.matmul(P[:, jg],
                                 bias_rows[0:1, dd, jg * 128:(jg + 1) * 128],
                                 onesrow[0:1, :nb], start=False, stop=False)
        for j in range(2):               # bh_n row opens the hpn region
            nc.tensor.matmul(P[:, 6 + j], bhn_row[0:1, dd, j * 128:(j + 1) * 128],
                             onesrow[0:1, :nb], start=True, stop=False)
        for j in range(2):               # xpn (x-side + bi_n)
            for k in range(nxk):
                nc.tensor.matmul(P[:, 8 + j],
                                 wi8[:, dd, k, 768 + j * 128:768 + (j + 1) * 128],
                                 xsrc(dd, k, t2), start=(k == 0),
                                 stop=(k == nxk - 1 and bias_rows is None))
            if bias_rows is not None:
                nc.tensor.matmul(P[:, 8 + j],
                                 bias_rows[0:1, dd, 768 + j * 128:768 + (j + 1) * 128],
                                 onesrow[0:1, :nb], start=False, stop=True)

    def emit_hp(dd, P):
        for jg in range(8):              # all gates from split state m1+m2
            for k in range(2):
                for mi, msrc in ((0, m1[dd]), (1, m2[dd])):
                    nc.tensor.matmul(P[:, jg],
                                     wh8[:, dd, k, jg * 128:(jg + 1) * 128],
                                     msrc[:, k, :], start=False,
                                     stop=(k == 1 and mi == 1))

    for t in range(LC):
        Pc = [None, None]
        for dd in range(2):
            t2 = t if dd == 0 else LC - 1 - t
            Pc[dd] = psg.tile([128, 10, nb], F32, tag=f"P{dd}_{nb}",
                              name=f"P{dd}_{nb}_{t}")
            emit_xp(dd, t2, Pc[dd])
        for dd in range(2):
            emit_hp(dd, Pc[dd])
        # critical path, both dirs in lockstep per stage:
        # sig1 -> r*hpn -> +xpn -> tanh -> zbar*n
        for dd in range(2):
            nc.scalar.activation(rzs[dd][:], Pc[dd][:, 0:6], AF.Sigmoid)
        for dd in range(2):
            nc.vector.tensor_tensor(narg[dd][:], rzs[dd][:, 0:2, :],
                                    Pc[dd][:, 6:8], ALU.mult)
        for dd in range(2):
            nc.vector.tensor_tensor(narg[dd][:], narg[dd][:], Pc[dd][:, 8:10],
                                    ALU.add)
        for dd in range(2):
            nc.scalar.activation(sst[dd][:], narg[dd][:], AF.Tanh)
        for dd in range(2):
            nc.vector.tensor_tensor(m2[dd][:], rzs[dd][:, 4:6, :], sst[dd][:],
                                    ALU.mult)
        # off critical path (SBUF-only, GpSimd)
        for dd in range(2):
            nc.gpsimd.tensor_tensor(m1[dd][:], rzs[dd][:, 2:4, :], gt[dd][:],
                                    ALU.mult)
        for dd in range(2):
            nc.vector.tensor_tensor(gt[dd][:], m1[dd][:], m2[dd][:], ALU.add)
            store(dd, t, gt[dd])


def _body(nc, tc, d):
    import contextlib
    ctx = contextlib.ExitStack()
    with ctx:
        consts = ctx.enter_context(tc.tile_pool(name="consts", bufs=1))
        wpool = ctx.enter_context(tc.tile_pool(name="weights", bufs=1))
        encp = ctx.enter_context(tc.tile_pool(name="enc", bufs=1))
        sbp = ctx.enter_context(tc.tile_pool(name="sbp", bufs=1))
        spool = ctx.enter_context(tc.tile_pool(name="spool", bufs=2))
        small = ctx.enter_context(tc.tile_pool(name="small", bufs=3))

        # ---- constants / weights ----
        ident = consts.tile([128, 128], F32)
        make_identity(nc, ident[:])
        ones128 = consts.tile([128, 1], F32)
        nc.vector.memset(ones128[:], 1.0)
        onesrow = consts.tile([1, NBA], BF16)
        nc.vector.memset(onesrow[:], 1.0)

        wi8m = wpool.tile([128, 2, 3, 1024], BF16)
        wh8m = wpool.tile([128, 2, 2, 1024], BF16)
        wi8a = wpool.tile([128, 2, 2, 1024], BF16)
        wh8a = wpool.tile([128, 2, 2, 1024], BF16)
        wk = wpool.tile([128, 4, H], BF16)
        wq = wpool.tile([128, 4, H], BF16)
        bhnm = consts.tile([1, 2, 256], BF16)
        bhna = consts.tile([1, 2, 256], BF16)
        batt = consts.tile([1, 2, 1024], BF16)
        vsb = consts.tile([128, 2], F32)
        for dd in range(2):
            for k in range(3):
                nc.sync.dma_start(wi8m[:, dd, k, :], d["wi8m"].ap()[dd, k])
            for k in range(2):
                nc.sync.dma_start(wh8m[:, dd, k, :], d["wh8m"].ap()[dd, k])
                nc.sync.dma_start(wi8a[:, dd, k, :], d["wi8a"].ap()[dd, k])
                nc.sync.dma_start(wh8a[:, dd, k, :], d["wh8a"].ap()[dd, k])
        for k in range(4):
            nc.sync.dma_start(wk[:, k, :], d["wk"].ap()[k])
            nc.sync.dma_start(wq[:, k, :], d["wq"].ap()[k])
        nc.sync.dma_start(bhnm[:], d["bhnm"].ap()[None])
        nc.sync.dma_start(bhna[:], d["bhna"].ap()[None])
        nc.sync.dma_start(batt[:], d["batt"].ap()[None])
        nc.sync.dma_start(vsb[:], d["v"].ap())

        xu = wpool.tile([128, 3, LC, NBM], BF16)
        for k in range(3):
            nc.sync.dma_start(xu[:, k, :, :], d["xu"].ap()[k])

        # ======== main GRU ========
        ence = encp.tile([128, 4, LC, BL], BF16)
        enco = encp.tile([128, 4, LO, NI], BF16)

        def store_main(dd, t, g):
            tc_ = t if dd == 0 else LC - 1 - t
            nc.gpsimd.tensor_copy(ence[:, 2 * dd:2 * dd + 2, tc_, :], g[:, :, 0:BL])
            to = t if dd == 0 else LO - 1 - t
            if 0 <= to < LO:
                nc.gpsimd.tensor_copy(enco[:, 2 * dd:2 * dd + 2, to, :], g[:, :, BL:])

        with tc.tile_pool(name="psgm", bufs=4, space="PSUM") as psg:
            _gru_loop(nc, tc, (psg, sbp), wh8m, wi8m,
                      lambda dd, k, t2: xu[:, k, t2, :], 3, None, bhnm,
                      onesrow, NBM, store_main)

        # ======== ctx_key / opt_q projections ========
        pse_cm = tc.tile_pool(name="pse", bufs=2, space="PSUM")
        pse = pse_cm.__enter__()
        ctxkT = encp.tile([128, 2, LC, BL], F32)
        optqT = encp.tile([128, 2, LO, NI], F32)

        def kq(dst, w, src, T, nb2, tch):
            for jg in range(2):
                for t0 in range(0, T, tch):
                    tw = min(tch, T - t0)
                    cw = tw * nb2
                    pt = pse.tile([128, 512], F32, tag="kq")
                    for k in range(4):
                        nc.tensor.matmul(
                            pt[:, :cw], w[:, k, jg * 128:(jg + 1) * 128],
                            src[:, k, t0:t0 + tw, :],
                            start=(k == 0), stop=(k == 3))
                    nc.vector.tensor_copy(dst[:, jg, t0:t0 + tw, :], pt[:, :cw])

        kq(ctxkT, wk, ence, LC, BL, 128)
        kq(optqT, wq, enco, LO, NI, 32)

        ctxk_cb = [[None, None] for _ in range(BL)]
        for b in range(BL):
            for jg in range(2):
                pt = pse.tile([128, 512], F32, tag="tr")
                nc.tensor.transpose(pt[:, :128], ctxkT[:, jg, :, b], ident[:])
                sb = small.tile([128, 128], BF16, tag=f"ck{b}{jg}")
                nc.vector.tensor_copy(sb[:], pt[:, :128])
                ctxk_cb[b][jg] = sb

        # ======== attention per (b, opt) ========
        # att-x unified tile: cols 0:10 actx, 10:20 aopt (dup at t 64:128)
        axu = encp.tile([128, 2, LC, NBA], BF16)
        QCH = 16
        for b in range(BL):
            for o in range(NOPT):
                i = b * NOPT + o
                e_ps = pse.tile([128, LO], F32, tag="e")
                for q0 in range(0, LO, QCH):
                    sts = []
                    for jg in range(2):
                        st = spool.tile([128, QCH, LC], F32, tag=f"s{jg}")
                        eng = nc.gpsimd if (q0 // QCH) % 3 == 2 else nc.vector
                        eng.tensor_tensor(
                            st[:],
                            optqT[:, jg, q0:q0 + QCH, i:i + 1]
                                .broadcast_to([128, QCH, LC]),
                            ctxkT[:, jg, None, :, b]
                                .broadcast_to([128, QCH, LC]),
                            ALU.add)
                        nc.scalar.activation(st[:], st[:], AF.Tanh)
                        sts.append(st)
                    for q in range(QCH):
                        for jg in range(2):
                            nc.tensor.matmul(
                                e_ps[:, q0 + q:q0 + q + 1],
                                sts[jg][:, q, :], vsb[:, jg:jg + 1],
                                start=(jg == 0), stop=(jg == 1))
                # softmax over q (free axis of e[c,q]) -> P1
                e_cq = small.tile([128, LO], F32, tag="ecq")
                nc.vector.tensor_copy(e_cq[:], e_ps[:])
                mx = small.tile([128, 1], F32, tag="mx")
                nc.vector.tensor_reduce(mx[:], e_cq[:],
                                        axis=mybir.AxisListType.X, op=ALU.max)
                nc.vector.tensor_scalar_mul(mx[:], mx[:], -1.0)
                p1 = small.tile([128, LO], F32, tag="p1")
                nc.scalar.activation(p1[:], e_cq[:], AF.Exp, bias=mx[:])
                sm = small.tile([128, 1], F32, tag="sm")
                nc.vector.tensor_reduce(sm[:], p1[:],
                                        axis=mybir.AxisListType.X, op=ALU.add)
                nc.vector.reciprocal(sm[:], sm[:])
                nc.vector.tensor_scalar_mul(p1[:], p1[:], sm[:])
                pt1 = pse.tile([128, 512], F32, tag="tr")
                nc.tensor.transpose(pt1[:64, :128], p1[:], ident[:])
                p1t = small.tile([64, 128], BF16, tag="p1tb")
                nc.vector.tensor_copy(p1t[:], pt1[:64, :128])
                # e^T -> softmax over c -> P2
                pt2 = pse.tile([128, 512], F32, tag="tr")
                nc.tensor.transpose(pt2[:64, :128], e_cq[:], ident[:])
                e_qc = small.tile([64, 128], F32, tag="eqc")
                nc.vector.tensor_copy(e_qc[:], pt2[:64, :128])
                mx2 = small.tile([64, 1], F32, tag="mx2")
                nc.vector.tensor_reduce(mx2[:], e_qc[:],
                                        axis=mybir.AxisListType.X, op=ALU.max)
                nc.vector.tensor_scalar_mul(mx2[:], mx2[:], -1.0)
                p2 = small.tile([64, 128], F32, tag="p2")
                nc.scalar.activation(p2[:], e_qc[:], AF.Exp, bias=mx2[:])
                sm2 = small.tile([64, 1], F32, tag="sm2")
                nc.vector.tensor_reduce(sm2[:], p2[:],
                                        axis=mybir.AxisListType.X, op=ALU.add)
                nc.vector.reciprocal(sm2[:], sm2[:])
                nc.vector.tensor_scalar_mul(p2[:], p2[:], sm2[:])
                pt3 = pse.tile([128, 512], F32, tag="tr")
                nc.tensor.transpose(pt3[:, :64], p2[:], ident[:64, :64])
                p2t = small.tile([128, 64], BF16, tag="p2tb")
                nc.vector.tensor_copy(p2t[:], pt3[:, :64])
                for jg in range(2):
                    pt4 = pse.tile([128, 512], F32, tag="tr")
                    nc.tensor.transpose(pt4[:64, :128], optqT[:, jg, :, i],
                                        ident[:])
                    oq = small.tile([64, 128], BF16, tag=f"oqb{jg}")
                    nc.vector.tensor_copy(oq[:], pt4[:64, :128])
                    ac_ps = pse.tile([128, 512], F32, tag="tr")
                    nc.tensor.matmul(ac_ps[:, :128], oq[:], p1t[:],
                                     start=True, stop=True)
                    nc.vector.tensor_copy(axu[:, jg, :, i], ac_ps[:, :128])
                    ao_ps = pse.tile([128, 512], F32, tag="tr")
                    nc.tensor.matmul(ao_ps[:, :64], ctxk_cb[b][jg][:], p2t[:],
                                     start=True, stop=True)
                    nc.vector.tensor_copy(axu[:, jg, 0:LO, NI + i], ao_ps[:, :64])
                    nc.gpsimd.tensor_copy(axu[:, jg, LO:LC, NI + i],
                                          axu[:, jg, 0:LO, NI + i])

        pse_cm.__exit__(None, None, None)

        # ======== att GRU with mean accumulation ========
        acc_c = encp.tile([128, 2, 2, NI], F32)
        acc_o = encp.tile([128, 2, 2, NI], F32)
        nc.vector.memset(acc_c[:], 0.0)
        nc.vector.memset(acc_o[:], 0.0)

        def store_att(dd, t, g):
            nc.vector.tensor_tensor(acc_c[:, dd], acc_c[:, dd],
                                    g[:, :, 0:NI], ALU.add)
            to = t if dd == 0 else LO - 1 - t
            if 0 <= to < LO:
                nc.vector.tensor_tensor(acc_o[:, dd], acc_o[:, dd],
                                        g[:, :, NI:], ALU.add)

        with tc.tile_pool(name="psga", bufs=4, space="PSUM") as psg:
            _gru_loop(nc, tc, (psg, sbp), wh8a, wi8a,
                      lambda dd, k, t2: axu[:, k, t2, :], 2, batt, bhna,
                      onesrow, NBA, store_att)

        pse = ctx.enter_context(tc.tile_pool(name="psec", bufs=1, space="PSUM"))

        # ======== cosine similarity ========
        nc.vector.tensor_scalar_mul(acc_c[:], acc_c[:], 1.0 / LC)
        nc.vector.tensor_scalar_mul(acc_o[:], acc_o[:], 1.0 / LO)
        prod = small.tile([128, 2, 2, NI], F32, tag="prod")
        dots_ps = pse.tile([1, 3, 4, NI], F32, tag="dots")
        nc.vector.tensor_tensor(prod[:], acc_c[:], acc_o[:], ALU.mult)
        nc.tensor.matmul(dots_ps[:, 0], ones128[:], prod[:],
                         start=True, stop=True)
        nc.vector.tensor_tensor(prod[:], acc_c[:], acc_c[:], ALU.mult)
        nc.tensor.matmul(dots_ps[:, 1], ones128[:], prod[:],
                         start=True, stop=True)
        nc.vector.tensor_tensor(prod[:], acc_o[:], acc_o[:], ALU.mult)
        nc.tensor.matmul(dots_ps[:, 2], ones128[:], prod[:],
                         start=True, stop=True)
        red = small.tile([1, 3, NI], F32, tag="red")
        nc.vector.tensor_reduce(red[:], dots_ps[:].transpose([0, 1, 3, 2]),
                                axis=mybir.AxisListType.X, op=ALU.add)
        nrm = small.tile([1, NI], F32, tag="nrm")
        nc.vector.tensor_tensor(nrm[:], red[:, 1, :], red[:, 2, :], ALU.mult)
        nc.vector.tensor_scalar_max(nrm[:], nrm[:], 1e-30)
        nc.scalar.activation(nrm[:], nrm[:], AF.Sqrt)
        nc.vector.reciprocal(nrm[:], nrm[:])
        cos = small.tile([1, NI], F32, tag="cos")
        nc.vector.tensor_tensor(cos[:], red[:, 0, :], nrm[:], ALU.mult)
        nc.sync.dma_start(d["out"].ap(), cos[:])


def _prep_inputs(inputs):
    ctx = np.asarray(inputs["context"], np.float32)
    opts = np.asarray(inputs["options"], np.float32)

    def gru_w(pre):
        out = {}
        for dd, sfx in enumerate(("f", "b")):
            out[dd] = {k: np.asarray(inputs[f"{pre}_{k}_{sfx}"], np.float32)
                       for k in ("Wi", "Wh", "bi", "bh")}
        return out

    rnn, att = gru_w("rnn"), gru_w("att")
    Wk = np.asarray(inputs["Wk"], np.float32)
    Wq = np.asarray(inputs["Wq"], np.float32)
    v = np.asarray(inputs["v_energy"], np.float32)

    def pack8_cols(W):  # W [3H, X] -> [X, 1024] gate-packed transpose
        WT = W.T  # [X, 3H]
        return np.concatenate(
            [WT[:, 0:H], WT[:, H:2 * H], -WT[:, H:2 * H], WT[:, 2 * H:]], axis=1)

    def wi8_pack(g, ein, with_bias_row):
        out = np.zeros((2, ((ein + 127) // 128) * 128 if not with_bias_row
                        else 384, 1024), np.float32)
        nk = out.shape[1] // 128
        for dd in range(2):
            m = pack8_cols(g[dd]["Wi"])  # [ein, 1024]
            out[dd, :ein] = m
            if with_bias_row:
                bi, bh = g[dd]["bi"], g[dd]["bh"]
                brow = np.concatenate([
                    bi[0:H] + bh[0:H], bi[H:2 * H] + bh[H:2 * H],
                    -(bi[H:2 * H] + bh[H:2 * H]), bi[2 * H:]])
                out[dd, ein] = brow
        return out.reshape(2, nk, 128, 1024).astype(bf)

    def wh8_pack(g):
        out = np.zeros((2, 256, 1024), np.float32)
        for dd in range(2):
            out[dd] = pack8_cols(g[dd]["Wh"])
        return out.reshape(2, 2, 128, 1024).astype(bf)

    def bhn_pack(g):
        out = np.zeros((2, 256), np.float32)
        for dd in range(2):
            out[dd] = g[dd]["bh"][2 * H:]
        return out.astype(bf)

    batt = np.zeros((2, 1024), np.float32)
    for dd in range(2):
        bi, bh = att[dd]["bi"], att[dd]["bh"]
        batt[dd] = np.concatenate([
            bi[0:H] + bh[0:H], bi[H:2 * H] + bh[H:2 * H],
            -(bi[H:2 * H] + bh[H:2 * H]), bi[2 * H:]])

    shared = {
        "wi8m": wi8_pack(rnn, E, True), "wh8m": wh8_pack(rnn),
        "wi8a": wi8_pack(att, H, False), "wh8a": wh8_pack(att),
        "bhnm": bhn_pack(rnn), "bhna": bhn_pack(att),
        "batt": batt.astype(bf),
        "wk": np.ascontiguousarray(Wk.T.reshape(4, 128, H).astype(bf)),
        "wq": np.ascontiguousarray(Wq.T.reshape(4, 128, H).astype(bf)),
        "v": np.ascontiguousarray(v.reshape(2, 128).T.astype(np.float32)),
    }

    in_maps = []
    for c in range(NCORES):
        bs = slice(c * BL, (c + 1) * BL)
        # unified x [384rows, 128t, 12cols]: ctx cols 0:2 (fwd order, 128t),
        # opt cols 2:12 duplicated at t 0:64 and 64:128
        xrows = np.zeros((384, LC, NBM), np.float32)
        xc = ctx[bs]                                  # [2, 128, 300]
        xo = opts[bs].reshape(NI, LO, E)              # [10, 64, 300]
        xrows[:E, :, 0:BL] = xc.transpose(2, 1, 0)
        xrows[E, :, 0:BL] = 1.0
        xrows[:E, 0:LO, BL:] = xo.transpose(2, 1, 0)
        xrows[:E, LO:, BL:] = xo.transpose(2, 1, 0)
        xrows[E, :, BL:] = 1.0
        m = dict(shared)
        m["xu"] = np.ascontiguousarray(
            xrows.reshape(3, 128, LC, NBM).reshape(3, 128, LC * NBM).astype(bf))
        in_maps.append(m)
    return in_maps


def kernel(**inputs):
    if "nc" not in _CACHE:
        _CACHE["nc"] = _build()
    nc = _CACHE["nc"]
    in_maps = _prep_inputs(inputs)
    res = bass_utils.run_bass_kernel_spmd(nc, in_maps,
                                          core_ids=list(range(NCORES)))
    _CACHE["last_exec_ns"] = res.exec_time_ns
    logits = np.concatenate(
        [np.asarray(res.results[c]["out"], np.float32).reshape(BL, NOPT)
         for c in range(NCORES)], axis=0)
    x = logits - logits.max(axis=1, keepdims=True)
    ex = np.exp(x)
    return (ex / ex.sum(axis=1, keepdims=True)).astype(np.float32)


if __name__ == "__main__":
    _build()
    print("build+compile OK")


# revision 26
# speedup vs baseline: 1.0114x; 1.0114x over previous
"""Bass/Trainium2 kernel for GruAttCosMeanNet (nn_GruAttCosMeanNet_39591008535146).

Data-parallel over batch: 8 cores x 2 batch rows each.

v2: latency-optimized GRU recurrences.
  - gates packed [r | z | zbar | n] (zbar = sigmoid of negated z-arg), so the
    update h' = z*h + zbar*n needs no (1-z) op.
  - split state h = m1 + m2 (m1 = z*h_prev off the critical path, m2 = zbar*n
    on it); the recurrence matmuls accumulate Wh*m1 + Wh*m2.
  - input projections computed inside the loop as PSUM-accumulated matmuls
    (x carries a ones-row so Wi biases fold in; bh_n enters via a 1-row
    matmul so it stays inside the r-gate product).
  - off-critical elementwise (m1, h-sum, enc stores, mean accumulators) on
    GpSimd; critical path: MM -> sig(r,z,zbar) -> r*hpn -> +xpn -> tanh ->
    zbar*n -> MM.
  - bwd direction reads the same fwd-ordered x via index T-1-t; opt cols
    (len 64) are stored twice (offset 0 and 64) so one index works for both.
"""
import sys
sys.path.insert(0, "/opt/trn_rl_repo")
import numpy as np
import ml_dtypes

import concourse.bass as bass
import concourse.mybir as mybir
import concourse.tile as tile
from concourse import bacc, bass_utils
from concourse.masks import make_identity

BF16 = mybir.dt.bfloat16
F32 = mybir.dt.float32
AF = mybir.ActivationFunctionType
ALU = mybir.AluOpType

B, LC, LO, NOPT, E, H = 16, 128, 64, 5, 300, 256
NCORES = 8
BL = B // NCORES          # 2 batch rows per core
NI = BL * NOPT            # 10 (b,opt) pairs per core
NBM = BL + NI             # 12 cols in main GRU (2 ctx + 10 opt)
NBA = 2 * NI              # 20 cols in att GRU (10 actx + 10 aopt)
H3 = 3 * H                # 768
bf = ml_dtypes.bfloat16

_CACHE = {}


def _build():
    nc = bacc.Bacc("TRN2", target_bir_lowering=False, debug=False,
                   num_devices=NCORES)

    d = {}
    # unified main x: [3k, 128, 128t*12cols] (ctx cols 0:2, opt cols 2:12;
    # opt rows duplicated at t 0:64 and 64:128)
    d["xu"] = nc.dram_tensor("xu", [3, 128, LC * NBM], BF16, kind="ExternalInput")
    # gate-packed weights: cols [r(256)|z(256)|zbar(256)|n(256)]
    d["wi8m"] = nc.dram_tensor("wi8m", [2, 3, 128, 1024], BF16, kind="ExternalInput")
    d["wh8m"] = nc.dram_tensor("wh8m", [2, 2, 128, 1024], BF16, kind="ExternalInput")
    d["wi8a"] = nc.dram_tensor("wi8a", [2, 2, 128, 1024], BF16, kind="ExternalInput")
    d["wh8a"] = nc.dram_tensor("wh8a", [2, 2, 128, 1024], BF16, kind="ExternalInput")
    d["bhnm"] = nc.dram_tensor("bhnm", [2, 256], BF16, kind="ExternalInput")  # [dir, n-cols]
    d["bhna"] = nc.dram_tensor("bhna", [2, 256], BF16, kind="ExternalInput")
    d["batt"] = nc.dram_tensor("batt", [2, 1024], BF16, kind="ExternalInput")  # att xp biases
    d["wk"] = nc.dram_tensor("wk", [4, 128, H], BF16, kind="ExternalInput")
    d["wq"] = nc.dram_tensor("wq", [4, 128, H], BF16, kind="ExternalInput")
    d["v"] = nc.dram_tensor("v", [128, 2], F32, kind="ExternalInput")
    d["out"] = nc.dram_tensor("out", [1, NI], F32, kind="ExternalOutput")

    with tile.TileContext(nc) as tc:
        _body(nc, tc, d)
    nc.compile()
    return nc


def _gru_loop(nc, tc, pools, wh8, wi8, xsrc, nxk, bias_rows, bhn_row, onesrow,
              nb, store, step_hook=None):
    """Shared bidirectional GRU loop, 128 steps.

    wh8: [128, 2dir, 2k, 1024] stationary (gate-packed).
    wi8: [128, 2dir, nxk, 1024] or per-dir-shared [128, nxk, 1024] (att).
    xsrc(d, k, t): moving AP [128, nb] for x input at fwd-index t.
    bias_rows: None or tile [1, 2dir, 1024] (att xp biases).
    bhn_row: [1, 2dir, 256] (bh_n, hp-side).
    store(d, t, g): store hidden state g [128, 2, nb] at step t.
    """
    psg, sbp = pools
    m1 = [None, None]
    m2 = [None, None]
    gt = [None, None]
    rzs = [None, None]
    narg = [None, None]
    sst = [None, None]
    for dd in range(2):
        m1[dd] = sbp.tile([128, 2, nb], BF16, tag=f"m1_{nb}_{dd}")
        m2[dd] = sbp.tile([128, 2, nb], BF16, tag=f"m2_{nb}_{dd}")
        gt[dd] = sbp.tile([128, 2, nb], BF16, tag=f"g_{nb}_{dd}")
        rzs[dd] = sbp.tile([128, 6, nb], F32, tag=f"rz_{nb}_{dd}")
        narg[dd] = sbp.tile([128, 2, nb], F32, tag=f"na_{nb}_{dd}")
        sst[dd] = sbp.tile([128, 2, nb], F32, tag=f"ss_{nb}_{dd}")
        nc.vector.memset(m1[dd][:], 0.0)
        nc.vector.memset(m2[dd][:], 0.0)
        nc.vector.memset(gt[dd][:], 0.0)

    def emit_xp(dd, t2, P):
        # x-side MMs + bias rows: no data deps, dispatch ahead
        for jg in range(6):
            for k in range(nxk):
                nc.tensor.matmul(P[:, jg], wi8[:, dd, k, jg * 128:(jg + 1) * 128],
                                 xsrc(dd, k, t2), start=(k == 0), stop=False)
            if bias_rows is not None:
                nc.tensor.matmul(P[:, jg],
                                 bias_rows[0:1, dd, jg * 128:(jg + 1) * 128],
                                 onesrow[0:1, :nb], start=False, stop=False)
        for j in range(2):               # bh_n row opens the hpn region
            nc.tensor.matmul(P[:, 6 + j], bhn_row[0:1, dd, j * 128:(j + 1) * 128],
                             onesrow[0:1, :nb], start=True, stop=False)
        for j in range(2):               # xpn (x-side + bi_n)
            for k in range(nxk):
                nc.tensor.matmul(P[:, 8 + j],
                                 wi8[:, dd, k, 768 + j * 128:768 + (j + 1) * 128],
                                 xsrc(dd, k, t2), start=(k == 0),
                                 stop=(k == nxk - 1 and bias_rows is None))
            if bias_rows is not None:
                nc.tensor.matmul(P[:, 8 + j],
                                 bias_rows[0:1, dd, 768 + j * 128:768 + (j + 1) * 128],
                                 onesrow[0:1, :nb], start=False, stop=True)

    def emit_hp(dd, P):
        for jg in range(8):              # all gates from split state m1+m2
            for k in range(2):
                for mi, msrc in ((0, m1[dd]), (1, m2[dd])):
                    nc.tensor.matmul(P[:, jg],
                                     wh8[:, dd, k, jg * 128:(jg + 1) * 128],
                                     msrc[:, k, :], start=False,
                                     stop=(k == 1 and mi == 1))

    for t in range(LC):
        Pc = [None, None]
        for dd in range(2):
            t2 = t if dd == 0 else LC - 1 - t
            Pc[dd] = psg.tile([128, 10, nb], F32, tag=f"P{dd}_{nb}",
                              name=f"P{dd}_{nb}_{t}")
            emit_xp(dd, t2, Pc[dd])
        for dd in range(2):
            emit_hp(dd, Pc[dd])
        # critical path, both dirs in lockstep per stage:
        # sig1 -> r*hpn -> +xpn -> tanh -> zbar*n
        for dd in range(2):
            nc.scalar.activation(rzs[dd][:], Pc[dd][:, 0:6], AF.Sigmoid)
        for dd in range(2):
            nc.vector.tensor_tensor(narg[dd][:], rzs[dd][:, 0:2, :],
                                    Pc[dd][:, 6:8], ALU.mult)
        for dd in range(2):
            nc.vector.tensor_tensor(narg[dd][:], narg[dd][:], Pc[dd][:, 8:10],
                                    ALU.add)
        for dd in range(2):
            nc.scalar.activation(sst[dd][:], narg[dd][:], AF.Tanh)
        for dd in range(2):
            nc.vector.tensor_tensor(m2[dd][:], rzs[dd][:, 4:6, :], sst[dd][:],
                                    ALU.mult)
        # off critical path (SBUF-only, GpSimd)
        for dd in range(2):
            nc.gpsimd.tensor_tensor(m1[dd][:], rzs[dd][:, 2:4, :], gt[dd][:],
                                    ALU.mult)
        for dd in range(2):
            nc.vector.tensor_tensor(gt[dd][:], m1[dd][:], m2[dd][:], ALU.add)
            store(dd, t, gt[dd])
        if step_hook is not None:
            step_hook(t)


def _body(nc, tc, d):
    import contextlib
    ctx = contextlib.ExitStack()
    with ctx:
        consts = ctx.enter_context(tc.tile_pool(name="consts", bufs=1))
        wpool = ctx.enter_context(tc.tile_pool(name="weights", bufs=1))
        encp = ctx.enter_context(tc.tile_pool(name="enc", bufs=1))
        sbp = ctx.enter_context(tc.tile_pool(name="sbp", bufs=1))
        spool = ctx.enter_context(tc.tile_pool(name="spool", bufs=2))
        small = ctx.enter_context(tc.tile_pool(name="small", bufs=3))

        # ---- constants / weights ----
        ident = consts.tile([128, 128], F32)
        make_identity(nc, ident[:])
        ones128 = consts.tile([128, 1], F32)
        nc.vector.memset(ones128[:], 1.0)
        onesrow = consts.tile([1, NBA], BF16)
        nc.vector.memset(onesrow[:], 1.0)

        wi8m = wpool.tile([128, 2, 3, 1024], BF16)
        wh8m = wpool.tile([128, 2, 2, 1024], BF16)
        wi8a = wpool.tile([128, 2, 2, 1024], BF16)
        wh8a = wpool.tile([128, 2, 2, 1024], BF16)
        wk = wpool.tile([128, 4, H], BF16)
        wq = wpool.tile([128, 4, H], BF16)
        bhnm = consts.tile([1, 2, 256], BF16)
        bhna = consts.tile([1, 2, 256], BF16)
        batt = consts.tile([1, 2, 1024], BF16)
        vsb = consts.tile([128, 2], F32)
        for dd in range(2):
            for k in range(3):
                nc.sync.dma_start(wi8m[:, dd, k, :], d["wi8m"].ap()[dd, k])
            for k in range(2):
                nc.sync.dma_start(wh8m[:, dd, k, :], d["wh8m"].ap()[dd, k])
        for k in range(4):
            nc.sync.dma_start(wk[:, k, :], d["wk"].ap()[k])
            nc.sync.dma_start(wq[:, k, :], d["wq"].ap()[k])
        nc.sync.dma_start(bhnm[:], d["bhnm"].ap()[None])
        nc.sync.dma_start(bhna[:], d["bhna"].ap()[None])
        nc.sync.dma_start(batt[:], d["batt"].ap()[None])
        nc.sync.dma_start(vsb[:], d["v"].ap())

        xu = wpool.tile([128, 3, LC, NBM], BF16)
        for k in range(3):
            nc.sync.dma_start(xu[:, k, :, :], d["xu"].ap()[k])
        for dd in range(2):
            for k in range(2):
                nc.sync.dma_start(wi8a[:, dd, k, :], d["wi8a"].ap()[dd, k])
                nc.sync.dma_start(wh8a[:, dd, k, :], d["wh8a"].ap()[dd, k])

        # ======== main GRU ========
        ence = encp.tile([128, 4, LC, BL], BF16)
        enco = encp.tile([128, 4, LO, NI], BF16)

        def store_main(dd, t, g):
            tc_ = t if dd == 0 else LC - 1 - t
            nc.gpsimd.tensor_copy(ence[:, 2 * dd:2 * dd + 2, tc_, :], g[:, :, 0:BL])
            to = t if dd == 0 else LO - 1 - t
            if 0 <= to < LO:
                nc.gpsimd.tensor_copy(enco[:, 2 * dd:2 * dd + 2, to, :], g[:, :, BL:])

        optqT = encp.tile([128, 2, LO, NI], F32)
        psk_cm = tc.tile_pool(name="psk", bufs=1, space="PSUM")
        psk = psk_cm.__enter__()

        def kq_chunk(dst, w, src, jg, t0, tw, nb2):
            cw = tw * nb2
            pt = psk.tile([128, 512], F32, tag="kqc", name="kqc")
            for k in range(4):
                nc.tensor.matmul(pt[:, :cw], w[:, k, jg * 128:(jg + 1) * 128],
                                 src[:, k, t0:t0 + tw, :],
                                 start=(k == 0), stop=(k == 3))
            nc.vector.tensor_copy(dst[:, jg, t0:t0 + tw, :], pt[:, :cw])

        # opt encoder finishes at main step 63; project opt_q during the tail
        def main_hook(t):
            if 68 <= t < 68 + 4 * 14 and (t - 68) % 14 == 0:
                ch = (t - 68) // 14
                kq_chunk(optqT, wq, enco, ch % 2, (ch // 2) * 32, 32, NI)

        with tc.tile_pool(name="psgm", bufs=3, space="PSUM") as psg:
            _gru_loop(nc, tc, (psg, sbp), wh8m, wi8m,
                      lambda dd, k, t2: xu[:, k, t2, :], 3, None, bhnm,
                      onesrow, NBM, store_main, step_hook=main_hook)

        psk_cm.__exit__(None, None, None)

        # ======== ctx_key / opt_q projections ========
        pse_cm = tc.tile_pool(name="pse", bufs=2, space="PSUM")
        pse = pse_cm.__enter__()
        ctxkT = encp.tile([128, 2, LC, BL], F32)

        def kq(dst, w, src, T, nb2, tch):
            for jg in range(2):
                for t0 in range(0, T, tch):
                    tw = min(tch, T - t0)
                    cw = tw * nb2
                    pt = pse.tile([128, 512], F32, tag="kq")
                    for k in range(4):
                        nc.tensor.matmul(
                            pt[:, :cw], w[:, k, jg * 128:(jg + 1) * 128],
                            src[:, k, t0:t0 + tw, :],
                            start=(k == 0), stop=(k == 3))
                    nc.vector.tensor_copy(dst[:, jg, t0:t0 + tw, :], pt[:, :cw])

        kq(ctxkT, wk, ence, LC, BL, 128)

        ctxk_cb = [[None, None] for _ in range(BL)]
        for b in range(BL):
            for jg in range(2):
                pt = pse.tile([128, 512], F32, tag="tr")
                nc.tensor.transpose(pt[:, :128], ctxkT[:, jg, :, b], ident[:])
                sb = small.tile([128, 128], BF16, tag=f"ck{b}{jg}")
                nc.vector.tensor_copy(sb[:], pt[:, :128])
                ctxk_cb[b][jg] = sb

        # ======== attention per (b, opt) ========
        # att-x unified tile: cols 0:10 actx, 10:20 aopt (dup at t 64:128)
        axu = encp.tile([128, 2, LC, NBA], BF16)
        QCH = 16
        for b in range(BL):
            for o in range(NOPT):
                i = b * NOPT + o
                e_ps = pse.tile([128, LO], F32, tag="e")
                for q0 in range(0, LO, QCH):
                    sts = []
                    for jg in range(2):
                        st = spool.tile([128, QCH, LC], F32, tag=f"s{jg}")
                        eng = nc.gpsimd if (q0 // QCH) % 3 == 2 else nc.vector
                        eng.tensor_tensor(
                            st[:],
                            optqT[:, jg, q0:q0 + QCH, i:i + 1]
                                .broadcast_to([128, QCH, LC]),
                            ctxkT[:, jg, None, :, b]
                                .broadcast_to([128, QCH, LC]),
                            ALU.add)
                        nc.scalar.activation(st[:], st[:], AF.Tanh)
                        sts.append(st)
                    for q in range(QCH):
                        for jg in range(2):
                            nc.tensor.matmul(
                                e_ps[:, q0 + q:q0 + q + 1],
                                sts[jg][:, q, :], vsb[:, jg:jg + 1],
                                start=(jg == 0), stop=(jg == 1))
                # softmax over q (free axis of e[c,q]) -> P1
                e_cq = small.tile([128, LO], F32, tag="ecq")
                nc.vector.tensor_copy(e_cq[:], e_ps[:])
                mx = small.tile([128, 1], F32, tag="mx")
                nc.vector.tensor_reduce(mx[:], e_cq[:],
                                        axis=mybir.AxisListType.X, op=ALU.max,
                                        negate=True)
                p1 = small.tile([128, LO], F32, tag="p1")
                nc.scalar.activation(p1[:], e_cq[:], AF.Exp, bias=mx[:])
                sm = small.tile([128, 1], F32, tag="sm")
                nc.vector.tensor_reduce(sm[:], p1[:],
                                        axis=mybir.AxisListType.X, op=ALU.add)
                nc.vector.reciprocal(sm[:], sm[:])
                nc.vector.tensor_scalar_mul(p1[:], p1[:], sm[:])
                pt1 = pse.tile([128, 512], F32, tag="tr")
                nc.tensor.transpose(pt1[:64, :128], p1[:], ident[:])
                p1t = small.tile([64, 128], BF16, tag="p1tb")
                nc.vector.tensor_copy(p1t[:], pt1[:64, :128])
                # e^T -> softmax over c -> P2
                pt2 = pse.tile([128, 512], F32, tag="tr")
                nc.tensor.transpose(pt2[:64, :128], e_cq[:], ident[:])
                e_qc = small.tile([64, 128], F32, tag="eqc")
                nc.vector.tensor_copy(e_qc[:], pt2[:64, :128])
                mx2 = small.tile([64, 1], F32, tag="mx2")
                nc.vector.tensor_reduce(mx2[:], e_qc[:],
                                        axis=mybir.AxisListType.X, op=ALU.max,
                                        negate=True)
                p2 = small.tile([64, 128], F32, tag="p2")
                nc.scalar.activation(p2[:], e_qc[:], AF.Exp, bias=mx2[:])
                sm2 = small.tile([64, 1], F32, tag="sm2")
                nc.vector.tensor_reduce(sm2[:], p2[:],
                                        axis=mybir.AxisListType.X, op=ALU.add)
                nc.vector.reciprocal(sm2[:], sm2[:])
                nc.vector.tensor_scalar_mul(p2[:], p2[:], sm2[:])
                pt3 = pse.tile([128, 512], F32, tag="tr")
                nc.tensor.transpose(pt3[:, :64], p2[:], ident[:64, :64])
                p2t = small.tile([128, 64], BF16, tag="p2tb")
                nc.vector.tensor_copy(p2t[:], pt3[:, :64])
                for jg in range(2):
                    pt4 = pse.tile([128, 512], F32, tag="tr")
                    nc.tensor.transpose(pt4[:64, :128], optqT[:, jg, :, i],
                                        ident[:])
                    oq = small.tile([64, 128], BF16, tag=f"oqb{jg}")
                    nc.vector.tensor_copy(oq[:], pt4[:64, :128])
                    ac_ps = pse.tile([128, 512], F32, tag="tr")
                    nc.tensor.matmul(ac_ps[:, :128], oq[:], p1t[:],
                                     start=True, stop=True)
                    nc.vector.tensor_copy(axu[:, jg, :, i], ac_ps[:, :128])
                    ao_ps = pse.tile([128, 512], F32, tag="tr")
                    nc.tensor.matmul(ao_ps[:, :64], ctxk_cb[b][jg][:], p2t[:],
                                     start=True, stop=True)
                    nc.vector.tensor_copy(axu[:, jg, 0:LO, NI + i], ao_ps[:, :64])
                    nc.gpsimd.tensor_copy(axu[:, jg, LO:LC, NI + i],
                                          axu[:, jg, 0:LO, NI + i])

        pse_cm.__exit__(None, None, None)

        # ======== att GRU with mean accumulation ========
        acc_c = encp.tile([128, 2, 2, NI], F32)
        acc_o = encp.tile([128, 2, 2, NI], F32)
        nc.vector.memset(acc_c[:], 0.0)
        nc.vector.memset(acc_o[:], 0.0)

        def store_att(dd, t, g):
            nc.gpsimd.tensor_tensor(acc_c[:, dd], acc_c[:, dd],
                                    g[:, :, 0:NI], ALU.add)
            to = t if dd == 0 else LO - 1 - t
            if 0 <= to < LO:
                nc.vector.tensor_tensor(acc_o[:, dd], acc_o[:, dd],
                                        g[:, :, NI:], ALU.add)

        with tc.tile_pool(name="psga", bufs=4, space="PSUM") as psg:
            _gru_loop(nc, tc, (psg, sbp), wh8a, wi8a,
                      lambda dd, k, t2: axu[:, k, t2, :], 2, batt, bhna,
                      onesrow, NBA, store_att)

        pse = ctx.enter_context(tc.tile_pool(name="psec", bufs=1, space="PSUM"))

        # ======== cosine similarity ========
        nc.vector.tensor_scalar_mul(acc_c[:], acc_c[:], 1.0 / LC)
        nc.vector.tensor_scalar_mul(acc_o[:], acc_o[:], 1.0 / LO)
        prod = small.tile([128, 2, 2, NI], F32, tag="prod")
        dots_ps = pse.tile([1, 3, 4, NI], F32, tag="dots")
        nc.vector.tensor_tensor(prod[:], acc_c[:], acc_o[:], ALU.mult)
        nc.tensor.matmul(dots_ps[:, 0], ones128[:], prod[:],
                         start=True, stop=True)
        nc.vector.tensor_tensor(prod[:], acc_c[:], acc_c[:], ALU.mult)
        nc.tensor.matmul(dots_ps[:, 1], ones128[:], prod[:],
                         start=True, stop=True)
        nc.vector.tensor_tensor(prod[:], acc_o[:], acc_o[:], ALU.mult)
        nc.tensor.matmul(dots_ps[:, 2], ones128[:], prod[:],
                         start=True, stop=True)
        red = small.tile([1, 3, NI], F32, tag="red")
        nc.vector.tensor_reduce(red[:], dots_ps[:].transpose([0, 1, 3, 2]),
                                axis=mybir.AxisListType.X, op=ALU.add)
        nrm = small.tile([1, NI], F32, tag="nrm")
        nc.vector.tensor_tensor(nrm[:], red[:, 1, :], red[:, 2, :], ALU.mult)
        nc.vector.tensor_scalar_max(nrm[:], nrm[:], 1e-30)
        nc.scalar.activation(nrm[:], nrm[:], AF.Sqrt)
        nc.vector.reciprocal(nrm[:], nrm[:])
        cos = small.tile([1, NI], F32, tag="cos")
        nc.vector.tensor_tensor(cos[:], red[:, 0, :], nrm[:], ALU.mult)
        nc.sync.dma_start(d["out"].ap(), cos[:])


def _prep_inputs(inputs):
    ctx = np.asarray(inputs["context"], np.float32)
    opts = np.asarray(inputs["options"], np.float32)

    def gru_w(pre):
        out = {}
        for dd, sfx in enumerate(("f", "b")):
            out[dd] = {k: np.asarray(inputs[f"{pre}_{k}_{sfx}"], np.float32)
                       for k in ("Wi", "Wh", "bi", "bh")}
        return out

    rnn, att = gru_w("rnn"), gru_w("att")
    Wk = np.asarray(inputs["Wk"], np.float32)
    Wq = np.asarray(inputs["Wq"], np.float32)
    v = np.asarray(inputs["v_energy"], np.float32)

    def pack8_cols(W):  # W [3H, X] -> [X, 1024] gate-packed transpose
        WT = W.T  # [X, 3H]
        return np.concatenate(
            [WT[:, 0:H], WT[:, H:2 * H], -WT[:, H:2 * H], WT[:, 2 * H:]], axis=1)

    def wi8_pack(g, ein, with_bias_row):
        out = np.zeros((2, ((ein + 127) // 128) * 128 if not with_bias_row
                        else 384, 1024), np.float32)
        nk = out.shape[1] // 128
        for dd in range(2):
            m = pack8_cols(g[dd]["Wi"])  # [ein, 1024]
            out[dd, :ein] = m
            if with_bias_row:
                bi, bh = g[dd]["bi"], g[dd]["bh"]
                brow = np.concatenate([
                    bi[0:H] + bh[0:H], bi[H:2 * H] + bh[H:2 * H],
                    -(bi[H:2 * H] + bh[H:2 * H]), bi[2 * H:]])
                out[dd, ein] = brow
        return out.reshape(2, nk, 128, 1024).astype(bf)

    def wh8_pack(g):
        out = np.zeros((2, 256, 1024), np.float32)
        for dd in range(2):
            out[dd] = pack8_cols(g[dd]["Wh"])
        return out.reshape(2, 2, 128, 1024).astype(bf)

    def bhn_pack(g):
        out = np.zeros((2, 256), np.float32)
        for dd in range(2):
            out[dd] = g[dd]["bh"][2 * H:]
        return out.astype(bf)

    batt = np.zeros((2, 1024), np.float32)
    for dd in range(2):
        bi, bh = att[dd]["bi"], att[dd]["bh"]
        batt[dd] = np.concatenate([
            bi[0:H] + bh[0:H], bi[H:2 * H] + bh[H:2 * H],
            -(bi[H:2 * H] + bh[H:2 * H]), bi[2 * H:]])

    shared = {
        "wi8m": wi8_pack(rnn, E, True), "wh8m": wh8_pack(rnn),
        "wi8a": wi8_pack(att, H, False), "wh8a": wh8_pack(att),
        "bhnm": bhn_pack(rnn), "bhna": bhn_pack(att),
        "batt": batt.astype(bf),
        "wk": np.ascontiguousarray(Wk.T.reshape(4, 128, H).astype(bf)),
        "wq": np.ascontiguousarray(Wq.T.reshape(4, 128, H).astype(bf)),
        "v": np.ascontiguousarray(v.reshape(2, 128).T.astype(np.float32)),
    }

    in_maps = []
    for c in range(NCORES):
        bs = slice(c * BL, (c + 1) * BL)
        # unified x [384rows, 128t, 12cols]: ctx cols 0:2 (fwd order, 128t),
        # opt cols 2:12 duplicated at t 0:64 and 64:128
        xrows = np.zeros((384, LC, NBM), np.float32)
        xc = ctx[bs]                                  # [2, 128, 300]
        xo = opts[bs].reshape(NI, LO, E)              # [10, 64, 300]
        xrows[:E, :, 0:BL] = xc.transpose(2, 1, 0)
        xrows[E, :, 0:BL] = 1.0
        xrows[:E, 0:LO, BL:] = xo.transpose(2, 1, 0)
        xrows[:E, LO:, BL:] = xo.transpose(2, 1, 0)
        xrows[E, :, BL:] = 1.0
        m = dict(shared)
        m["xu"] = np.ascontiguousarray(
            xrows.reshape(3, 128, LC, NBM).reshape(3, 128, LC * NBM).astype(bf))
        in_maps.append(m)
    return in_maps


def kernel(**inputs):
    if "nc" not in _CACHE:
        _CACHE["nc"] = _build()
    nc = _CACHE["nc"]
    in_maps = _prep_inputs(inputs)
    res = bass_utils.run_bass_kernel_spmd(nc, in_maps,
                                          core_ids=list(range(NCORES)))
    _CACHE["last_exec_ns"] = res.exec_time_ns
    logits = np.concatenate(
        [np.asarray(res.results[c]["out"], np.float32).reshape(BL, NOPT)
         for c in range(NCORES)], axis=0)
    x = logits - logits.max(axis=1, keepdims=True)
    ex = np.exp(x)
    return (ex / ex.sum(axis=1, keepdims=True)).astype(np.float32)


if __name__ == "__main__":
    _build()
    print("build+compile OK")


# revision 29
# speedup vs baseline: 1.0161x; 1.0047x over previous
"""Bass/Trainium2 kernel for GruAttCosMeanNet (nn_GruAttCosMeanNet_39591008535146).

Data-parallel over batch: 8 cores x 2 batch rows each.

v2: latency-optimized GRU recurrences.
  - gates packed [r | z | zbar | n] (zbar = sigmoid of negated z-arg), so the
    update h' = z*h + zbar*n needs no (1-z) op.
  - split state h = m1 + m2 (m1 = z*h_prev off the critical path, m2 = zbar*n
    on it); the recurrence matmuls accumulate Wh*m1 + Wh*m2.
  - input projections computed inside the loop as PSUM-accumulated matmuls
    (x carries a ones-row so Wi biases fold in; bh_n enters via a 1-row
    matmul so it stays inside the r-gate product).
  - off-critical elementwise (m1, h-sum, enc stores, mean accumulators) on
    GpSimd; critical path: MM -> sig(r,z,zbar) -> r*hpn -> +xpn -> tanh ->
    zbar*n -> MM.
  - bwd direction reads the same fwd-ordered x via index T-1-t; opt cols
    (len 64) are stored twice (offset 0 and 64) so one index works for both.
"""
import sys
sys.path.insert(0, "/opt/trn_rl_repo")
import numpy as np
import ml_dtypes

import concourse.bass as bass
import concourse.mybir as mybir
import concourse.tile as tile
from concourse import bacc, bass_utils
from concourse.masks import make_identity

BF16 = mybir.dt.bfloat16
F32 = mybir.dt.float32
AF = mybir.ActivationFunctionType
ALU = mybir.AluOpType

B, LC, LO, NOPT, E, H = 16, 128, 64, 5, 300, 256
NCORES = 8
BL = B // NCORES          # 2 batch rows per core
NI = BL * NOPT            # 10 (b,opt) pairs per core
NBM = BL + NI             # 12 cols in main GRU (2 ctx + 10 opt)
NBA = 2 * NI              # 20 cols in att GRU (10 actx + 10 aopt)
H3 = 3 * H                # 768
bf = ml_dtypes.bfloat16

_CACHE = {}


def _build():
    nc = bacc.Bacc("TRN2", target_bir_lowering=False, debug=False,
                   num_devices=NCORES)

    d = {}
    # unified main x: [3k, 128, 128t*12cols] (ctx cols 0:2, opt cols 2:12;
    # opt rows duplicated at t 0:64 and 64:128)
    d["xu"] = nc.dram_tensor("xu", [3, 128, LC * NBM], BF16, kind="ExternalInput")
    # gate-packed weights: cols [r(256)|z(256)|zbar(256)|n(256)]
    d["wi8m"] = nc.dram_tensor("wi8m", [2, 3, 128, 1024], BF16, kind="ExternalInput")
    d["wh8m"] = nc.dram_tensor("wh8m", [2, 2, 128, 1024], BF16, kind="ExternalInput")
    d["wi8a"] = nc.dram_tensor("wi8a", [2, 2, 128, 1024], BF16, kind="ExternalInput")
    d["wh8a"] = nc.dram_tensor("wh8a", [2, 2, 128, 1024], BF16, kind="ExternalInput")
    d["bhnm"] = nc.dram_tensor("bhnm", [2, 256], BF16, kind="ExternalInput")  # [dir, n-cols]
    d["bhna"] = nc.dram_tensor("bhna", [2, 256], BF16, kind="ExternalInput")
    d["batt"] = nc.dram_tensor("batt", [2, 1024], BF16, kind="ExternalInput")  # att xp biases
    d["wk"] = nc.dram_tensor("wk", [4, 128, H], BF16, kind="ExternalInput")
    d["wq"] = nc.dram_tensor("wq", [4, 128, H], BF16, kind="ExternalInput")
    d["v"] = nc.dram_tensor("v", [128, 2], F32, kind="ExternalInput")
    d["out"] = nc.dram_tensor("out", [1, NI], F32, kind="ExternalOutput")

    with tile.TileContext(nc) as tc:
        _body(nc, tc, d)
    nc.compile()
    return nc


def _gru_loop(nc, tc, pools, wh8, wi8, xsrc, nxk, bias_rows, bhn_row, onesrow,
              nb, store, step_hook=None):
    """Shared bidirectional GRU loop, 128 steps.

    wh8: [128, 2dir, 2k, 1024] stationary (gate-packed).
    wi8: [128, 2dir, nxk, 1024] or per-dir-shared [128, nxk, 1024] (att).
    xsrc(d, k, t): moving AP [128, nb] for x input at fwd-index t.
    bias_rows: None or tile [1, 2dir, 1024] (att xp biases).
    bhn_row: [1, 2dir, 256] (bh_n, hp-side).
    store(d, t, g): store hidden state g [128, 2, nb] at step t.
    """
    psg, sbp = pools
    m1 = [None, None]
    m2 = [None, None]
    gt = [None, None]
    rzs = [None, None]
    narg = [None, None]
    sst = [None, None]
    for dd in range(2):
        m1[dd] = sbp.tile([128, 2, nb], BF16, tag=f"m1_{nb}_{dd}")
        m2[dd] = sbp.tile([128, 2, nb], BF16, tag=f"m2_{nb}_{dd}")
        gt[dd] = sbp.tile([128, 2, nb], BF16, tag=f"g_{nb}_{dd}")
        rzs[dd] = sbp.tile([128, 6, nb], F32, tag=f"rz_{nb}_{dd}")
        narg[dd] = sbp.tile([128, 2, nb], F32, tag=f"na_{nb}_{dd}")
        sst[dd] = sbp.tile([128, 2, nb], F32, tag=f"ss_{nb}_{dd}")
        nc.vector.memset(m1[dd][:], 0.0)
        nc.vector.memset(m2[dd][:], 0.0)
        nc.vector.memset(gt[dd][:], 0.0)

    def emit_xp(dd, t2, P):
        # x-side MMs + bias rows: no data deps, dispatch ahead
        for jg in range(6):
            for k in range(nxk):
                nc.tensor.matmul(P[:, jg], wi8[:, dd, k, jg * 128:(jg + 1) * 128],
                                 xsrc(dd, k, t2), start=(k == 0), stop=False)
            if bias_rows is not None:
                nc.tensor.matmul(P[:, jg],
                                 bias_rows[0:1, dd, jg * 128:(jg + 1) * 128],
                                 onesrow[0:1, :nb], start=False, stop=False)
        for j in range(2):               # bh_n row opens the hpn region
            nc.tensor.matmul(P[:, 6 + j], bhn_row[0:1, dd, j * 128:(j + 1) * 128],
                             onesrow[0:1, :nb], start=True, stop=False)
        for j in range(2):               # xpn (x-side + bi_n)
            for k in range(nxk):
                nc.tensor.matmul(P[:, 8 + j],
                                 wi8[:, dd, k, 768 + j * 128:768 + (j + 1) * 128],
                                 xsrc(dd, k, t2), start=(k == 0),
                                 stop=(k == nxk - 1 and bias_rows is None))
            if bias_rows is not None:
                nc.tensor.matmul(P[:, 8 + j],
                                 bias_rows[0:1, dd, 768 + j * 128:768 + (j + 1) * 128],
                                 onesrow[0:1, :nb], start=False, stop=True)

    def emit_hp(dd, P):
        for jg in range(8):              # all gates from split state m1+m2
            for k in range(2):
                for mi, msrc in ((0, m1[dd]), (1, m2[dd])):
                    nc.tensor.matmul(P[:, jg],
                                     wh8[:, dd, k, jg * 128:(jg + 1) * 128],
                                     msrc[:, k, :], start=False,
                                     stop=(k == 1 and mi == 1))

    for t in range(LC):
        Pc = [None, None]
        for dd in range(2):
            t2 = t if dd == 0 else LC - 1 - t
            Pc[dd] = psg.tile([128, 10, nb], F32, tag=f"P{dd}_{nb}",
                              name=f"P{dd}_{nb}_{t}")
            emit_xp(dd, t2, Pc[dd])
        for dd in range(2):
            emit_hp(dd, Pc[dd])
        # critical path, both dirs in lockstep per stage:
        # sig1 -> r*hpn -> +xpn -> tanh -> zbar*n
        for dd in range(2):
            nc.scalar.activation(rzs[dd][:], Pc[dd][:, 0:6], AF.Sigmoid)
        for dd in range(2):
            nc.vector.tensor_tensor(narg[dd][:], rzs[dd][:, 0:2, :],
                                    Pc[dd][:, 6:8], ALU.mult)
        for dd in range(2):
            nc.vector.tensor_tensor(narg[dd][:], narg[dd][:], Pc[dd][:, 8:10],
                                    ALU.add)
        for dd in range(2):
            nc.scalar.activation(sst[dd][:], narg[dd][:], AF.Tanh)
        for dd in range(2):
            nc.vector.tensor_tensor(m2[dd][:], rzs[dd][:, 4:6, :], sst[dd][:],
                                    ALU.mult)
        # off critical path (SBUF-only, GpSimd)
        for dd in range(2):
            nc.gpsimd.tensor_tensor(m1[dd][:], rzs[dd][:, 2:4, :], gt[dd][:],
                                    ALU.mult)
        for dd in range(2):
            nc.vector.tensor_tensor(gt[dd][:], m1[dd][:], m2[dd][:], ALU.add)
            store(dd, t, gt[dd])
        if step_hook is not None:
            step_hook(t)


def _body(nc, tc, d):
    import contextlib
    ctx = contextlib.ExitStack()
    with ctx:
        consts = ctx.enter_context(tc.tile_pool(name="consts", bufs=1))
        wpool = ctx.enter_context(tc.tile_pool(name="weights", bufs=1))
        encp = ctx.enter_context(tc.tile_pool(name="enc", bufs=1))
        sbp = ctx.enter_context(tc.tile_pool(name="sbp", bufs=1))
        spool = ctx.enter_context(tc.tile_pool(name="spool", bufs=2))
        small = ctx.enter_context(tc.tile_pool(name="small", bufs=3))

        # ---- constants / weights ----
        ident = consts.tile([128, 128], F32)
        make_identity(nc, ident[:])
        ones128 = consts.tile([128, 1], F32)
        nc.vector.memset(ones128[:], 1.0)
        onesrow = consts.tile([1, NBA], BF16)
        nc.vector.memset(onesrow[:], 1.0)

        wi8m = wpool.tile([128, 2, 3, 1024], BF16)
        wh8m = wpool.tile([128, 2, 2, 1024], BF16)
        wi8a = wpool.tile([128, 2, 2, 1024], BF16)
        wh8a = wpool.tile([128, 2, 2, 1024], BF16)
        wk = wpool.tile([128, 4, H], BF16)
        wq = wpool.tile([128, 4, H], BF16)
        bhnm = consts.tile([1, 2, 256], BF16)
        bhna = consts.tile([1, 2, 256], BF16)
        batt = consts.tile([1, 2, 1024], BF16)
        vsb = consts.tile([128, 2], F32)
        for dd in range(2):
            for k in range(3):
                nc.sync.dma_start(wi8m[:, dd, k, :], d["wi8m"].ap()[dd, k])
            for k in range(2):
                nc.sync.dma_start(wh8m[:, dd, k, :], d["wh8m"].ap()[dd, k])
        for k in range(4):
            nc.sync.dma_start(wk[:, k, :], d["wk"].ap()[k])
            nc.sync.dma_start(wq[:, k, :], d["wq"].ap()[k])
        nc.sync.dma_start(bhnm[:], d["bhnm"].ap()[None])
        nc.sync.dma_start(bhna[:], d["bhna"].ap()[None])
        nc.sync.dma_start(batt[:], d["batt"].ap()[None])
        nc.sync.dma_start(vsb[:], d["v"].ap())

        xu = wpool.tile([128, 3, LC, NBM], BF16)
        for k in range(3):
            nc.sync.dma_start(xu[:, k, :, :], d["xu"].ap()[k])
        for dd in range(2):
            for k in range(2):
                nc.sync.dma_start(wi8a[:, dd, k, :], d["wi8a"].ap()[dd, k])
                nc.sync.dma_start(wh8a[:, dd, k, :], d["wh8a"].ap()[dd, k])

        # ======== main GRU ========
        ence = encp.tile([128, 4, LC, BL], BF16)
        enco = encp.tile([128, 4, LO, NI], BF16)

        def store_main(dd, t, g):
            tc_ = t if dd == 0 else LC - 1 - t
            nc.gpsimd.tensor_copy(ence[:, 2 * dd:2 * dd + 2, tc_, :], g[:, :, 0:BL])
            to = t if dd == 0 else LO - 1 - t
            if 0 <= to < LO:
                nc.gpsimd.tensor_copy(enco[:, 2 * dd:2 * dd + 2, to, :], g[:, :, BL:])

        optqT = encp.tile([128, 2, LO, NI], F32)
        psk_cm = tc.tile_pool(name="psk", bufs=1, space="PSUM")
        psk = psk_cm.__enter__()

        def kq_chunk(dst, w, src, jg, t0, tw, nb2):
            cw = tw * nb2
            pt = psk.tile([128, 512], F32, tag="kqc", name="kqc")
            for k in range(4):
                nc.tensor.matmul(pt[:, :cw], w[:, k, jg * 128:(jg + 1) * 128],
                                 src[:, k, t0:t0 + tw, :],
                                 start=(k == 0), stop=(k == 3))
            nc.vector.tensor_copy(dst[:, jg, t0:t0 + tw, :], pt[:, :cw])

        oqs = [[None, None] for _ in range(NI)]
        for _i in range(NI):
            for _jg in range(2):
                oqs[_i][_jg] = small.tile([64, 128], BF16, tag=f"oq{_i}_{_jg}",
                                          name=f"oq{_i}_{_jg}")

        # opt encoder finishes at main step 63: project opt_q, then
        # transpose it for the attention aggregation, all during the tail
        def main_hook(t):
            if 68 <= t < 68 + 4 * 14 and (t - 68) % 14 == 0:
                ch = (t - 68) // 14
                kq_chunk(optqT, wq, enco, ch % 2, (ch // 2) * 32, 32, NI)
            if 100 <= t < 110 or 117 <= t < 127:
                i, jg = (t - 100, 0) if t < 110 else (t - 117, 1)
                pt4 = psk.tile([128, 512], F32, tag="kqc", name=f"oqt{i}{jg}")
                nc.tensor.transpose(pt4[:64, :128], optqT[:, jg, :, i],
                                    ident[:])
                nc.vector.tensor_copy(oqs[i][jg][:], pt4[:64, :128])

        with tc.tile_pool(name="psgm", bufs=3, space="PSUM") as psg:
            _gru_loop(nc, tc, (psg, sbp), wh8m, wi8m,
                      lambda dd, k, t2: xu[:, k, t2, :], 3, None, bhnm,
                      onesrow, NBM, store_main, step_hook=main_hook)

        psk_cm.__exit__(None, None, None)

        # ======== ctx_key / opt_q projections ========
        pse_cm = tc.tile_pool(name="pse", bufs=2, space="PSUM")
        pse = pse_cm.__enter__()
        ctxkT = encp.tile([128, 2, LC, BL], F32)

        def kq(dst, w, src, T, nb2, tch):
            for jg in range(2):
                for t0 in range(0, T, tch):
                    tw = min(tch, T - t0)
                    cw = tw * nb2
                    pt = pse.tile([128, 512], F32, tag="kq")
                    for k in range(4):
                        nc.tensor.matmul(
                            pt[:, :cw], w[:, k, jg * 128:(jg + 1) * 128],
                            src[:, k, t0:t0 + tw, :],
                            start=(k == 0), stop=(k == 3))
                    nc.vector.tensor_copy(dst[:, jg, t0:t0 + tw, :], pt[:, :cw])

        kq(ctxkT, wk, ence, LC, BL, 128)

        ctxk_cb = [[None, None] for _ in range(BL)]
        for b in range(BL):
            for jg in range(2):
                pt = pse.tile([128, 512], F32, tag="tr")
                nc.tensor.transpose(pt[:, :128], ctxkT[:, jg, :, b], ident[:])
                sb = small.tile([128, 128], BF16, tag=f"ck{b}{jg}")
                nc.vector.tensor_copy(sb[:], pt[:, :128])
                ctxk_cb[b][jg] = sb

        # ======== attention per (b, opt) ========
        # att-x unified tile: cols 0:10 actx, 10:20 aopt (dup at t 64:128)
        axu = encp.tile([128, 2, LC, NBA], BF16)
        QCH = 16
        for b in range(BL):
            for o in range(NOPT):
                i = b * NOPT + o
                e_ps = pse.tile([128, LO], F32, tag="e")
                for q0 in range(0, LO, QCH):
                    sts = []
                    for jg in range(2):
                        st = spool.tile([128, QCH, LC], F32, tag=f"s{jg}")
                        eng = nc.gpsimd if (q0 // QCH) % 3 == 2 else nc.vector
                        eng.tensor_tensor(
                            st[:],
                            optqT[:, jg, q0:q0 + QCH, i:i + 1]
                                .broadcast_to([128, QCH, LC]),
                            ctxkT[:, jg, None, :, b]
                                .broadcast_to([128, QCH, LC]),
                            ALU.add)
                        nc.scalar.activation(st[:], st[:], AF.Tanh)
                        sts.append(st)
                    for q in range(QCH):
                        for jg in range(2):
                            nc.tensor.matmul(
                                e_ps[:, q0 + q:q0 + q + 1],
                                sts[jg][:, q, :], vsb[:, jg:jg + 1],
                                start=(jg == 0), stop=(jg == 1))
                # softmax over q (free axis of e[c,q]) -> P1
                e_cq = small.tile([128, LO], F32, tag="ecq")
                nc.vector.tensor_copy(e_cq[:], e_ps[:])
                mx = small.tile([128, 1], F32, tag="mx")
                nc.vector.tensor_reduce(mx[:], e_cq[:],
                                        axis=mybir.AxisListType.X, op=ALU.max,
                                        negate=True)
                p1 = small.tile([128, LO], F32, tag="p1")
                nc.scalar.activation(p1[:], e_cq[:], AF.Exp, bias=mx[:])
                sm = small.tile([128, 1], F32, tag="sm")
                nc.vector.tensor_reduce(sm[:], p1[:],
                                        axis=mybir.AxisListType.X, op=ALU.add)
                nc.vector.reciprocal(sm[:], sm[:])
                nc.vector.tensor_scalar_mul(p1[:], p1[:], sm[:])
                pt1 = pse.tile([128, 512], F32, tag="tr")
                nc.tensor.transpose(pt1[:64, :128], p1[:], ident[:])
                p1t = small.tile([64, 128], BF16, tag="p1tb")
                nc.vector.tensor_copy(p1t[:], pt1[:64, :128])
                # e^T -> softmax over c -> P2
                pt2 = pse.tile([128, 512], F32, tag="tr")
                nc.tensor.transpose(pt2[:64, :128], e_cq[:], ident[:])
                e_qc = small.tile([64, 128], F32, tag="eqc")
                nc.vector.tensor_copy(e_qc[:], pt2[:64, :128])
                mx2 = small.tile([64, 1], F32, tag="mx2")
                nc.vector.tensor_reduce(mx2[:], e_qc[:],
                                        axis=mybir.AxisListType.X, op=ALU.max,
                                        negate=True)
                p2 = small.tile([64, 128], F32, tag="p2")
                nc.scalar.activation(p2[:], e_qc[:], AF.Exp, bias=mx2[:])
                sm2 = small.tile([64, 1], F32, tag="sm2")
                nc.vector.tensor_reduce(sm2[:], p2[:],
                                        axis=mybir.AxisListType.X, op=ALU.add)
                nc.vector.reciprocal(sm2[:], sm2[:])
                nc.vector.tensor_scalar_mul(p2[:], p2[:], sm2[:])
                pt3 = pse.tile([128, 512], F32, tag="tr")
                nc.tensor.transpose(pt3[:, :64], p2[:], ident[:64, :64])
                p2t = small.tile([128, 64], BF16, tag="p2tb")
                nc.vector.tensor_copy(p2t[:], pt3[:, :64])
                for jg in range(2):
                    oq = oqs[i][jg]
                    ac_ps = pse.tile([128, 512], F32, tag="tr")
                    nc.tensor.matmul(ac_ps[:, :128], oq[:], p1t[:],
                                     start=True, stop=True)
                    nc.vector.tensor_copy(axu[:, jg, :, i], ac_ps[:, :128])
                    ao_ps = pse.tile([128, 512], F32, tag="tr")
                    nc.tensor.matmul(ao_ps[:, :64], ctxk_cb[b][jg][:], p2t[:],
                                     start=True, stop=True)
                    nc.vector.tensor_copy(axu[:, jg, 0:LO, NI + i], ao_ps[:, :64])
                    nc.gpsimd.tensor_copy(axu[:, jg, LO:LC, NI + i],
                                          axu[:, jg, 0:LO, NI + i])

        pse_cm.__exit__(None, None, None)

        # ======== att GRU with mean accumulation ========
        acc_c = encp.tile([128, 2, 2, NI], F32)
        acc_o = encp.tile([128, 2, 2, NI], F32)
        nc.vector.memset(acc_c[:], 0.0)
        nc.vector.memset(acc_o[:], 0.0)

        def store_att(dd, t, g):
            nc.gpsimd.tensor_tensor(acc_c[:, dd], acc_c[:, dd],
                                    g[:, :, 0:NI], ALU.add)
            to = t if dd == 0 else LO - 1 - t
            if 0 <= to < LO:
                nc.vector.tensor_tensor(acc_o[:, dd], acc_o[:, dd],
                                        g[:, :, NI:], ALU.add)

        with tc.tile_pool(name="psga", bufs=4, space="PSUM") as psg:
            _gru_loop(nc, tc, (psg, sbp), wh8a, wi8a,
                      lambda dd, k, t2: axu[:, k, t2, :], 2, batt, bhna,
                      onesrow, NBA, store_att)

        pse = ctx.enter_context(tc.tile_pool(name="psec", bufs=1, space="PSUM"))

        # ======== cosine similarity ========
        nc.vector.tensor_scalar_mul(acc_c[:], acc_c[:], 1.0 / LC)
        nc.vector.tensor_scalar_mul(acc_o[:], acc_o[:], 1.0 / LO)
        prod = small.tile([128, 2, 2, NI], F32, tag="prod")
        dots_ps = pse.tile([1, 3, 4, NI], F32, tag="dots")
        nc.vector.tensor_tensor(prod[:], acc_c[:], acc_o[:], ALU.mult)
        nc.tensor.matmul(dots_ps[:, 0], ones128[:], prod[:],
                         start=True, stop=True)
        nc.vector.tensor_tensor(prod[:], acc_c[:], acc_c[:], ALU.mult)
        nc.tensor.matmul(dots_ps[:, 1], ones128[:], prod[:],
                         start=True, stop=True)
        nc.vector.tensor_tensor(prod[:], acc_o[:], acc_o[:], ALU.mult)
        nc.tensor.matmul(dots_ps[:, 2], ones128[:], prod[:],
                         start=True, stop=True)
        red = small.tile([1, 3, NI], F32, tag="red")
        nc.vector.tensor_reduce(red[:], dots_ps[:].transpose([0, 1, 3, 2]),
                                axis=mybir.AxisListType.X, op=ALU.add)
        nrm = small.tile([1, NI], F32, tag="nrm")
        nc.vector.tensor_tensor(nrm[:], red[:, 1, :], red[:, 2, :], ALU.mult)
        nc.vector.tensor_scalar_max(nrm[:], nrm[:], 1e-30)
        nc.scalar.activation(nrm[:], nrm[:], AF.Sqrt)
        nc.vector.reciprocal(nrm[:], nrm[:])
        cos = small.tile([1, NI], F32, tag="cos")
        nc.vector.tensor_tensor(cos[:], red[:, 0, :], nrm[:], ALU.mult)
        nc.sync.dma_start(d["out"].ap(), cos[:])


def _prep_inputs(inputs):
    ctx = np.asarray(inputs["context"], np.float32)
    opts = np.asarray(inputs["options"], np.float32)

    def gru_w(pre):
        out = {}
        for dd, sfx in enumerate(("f", "b")):
            out[dd] = {k: np.asarray(inputs[f"{pre}_{k}_{sfx}"], np.float32)
                       for k in ("Wi", "Wh", "bi", "bh")}
        return out

    rnn, att = gru_w("rnn"), gru_w("att")
    Wk = np.asarray(inputs["Wk"], np.float32)
    Wq = np.asarray(inputs["Wq"], np.float32)
    v = np.asarray(inputs["v_energy"], np.float32)

    def pack8_cols(W):  # W [3H, X] -> [X, 1024] gate-packed transpose
        WT = W.T  # [X, 3H]
        return np.concatenate(
            [WT[:, 0:H], WT[:, H:2 * H], -WT[:, H:2 * H], WT[:, 2 * H:]], axis=1)

    def wi8_pack(g, ein, with_bias_row):
        out = np.zeros((2, ((ein + 127) // 128) * 128 if not with_bias_row
                        else 384, 1024), np.float32)
        nk = out.shape[1] // 128
        for dd in range(2):
            m = pack8_cols(g[dd]["Wi"])  # [ein, 1024]
            out[dd, :ein] = m
            if with_bias_row:
                bi, bh = g[dd]["bi"], g[dd]["bh"]
                brow = np.concatenate([
                    bi[0:H] + bh[0:H], bi[H:2 * H] + bh[H:2 * H],
                    -(bi[H:2 * H] + bh[H:2 * H]), bi[2 * H:]])
                out[dd, ein] = brow
        return out.reshape(2, nk, 128, 1024).astype(bf)

    def wh8_pack(g):
        out = np.zeros((2, 256, 1024), np.float32)
        for dd in range(2):
            out[dd] = pack8_cols(g[dd]["Wh"])
        return out.reshape(2, 2, 128, 1024).astype(bf)

    def bhn_pack(g):
        out = np.zeros((2, 256), np.float32)
        for dd in range(2):
            out[dd] = g[dd]["bh"][2 * H:]
        return out.astype(bf)

    batt = np.zeros((2, 1024), np.float32)
    for dd in range(2):
        bi, bh = att[dd]["bi"], att[dd]["bh"]
        batt[dd] = np.concatenate([
            bi[0:H] + bh[0:H], bi[H:2 * H] + bh[H:2 * H],
            -(bi[H:2 * H] + bh[H:2 * H]), bi[2 * H:]])

    shared = {
        "wi8m": wi8_pack(rnn, E, True), "wh8m": wh8_pack(rnn),
        "wi8a": wi8_pack(att, H, False), "wh8a": wh8_pack(att),
        "bhnm": bhn_pack(rnn), "bhna": bhn_pack(att),
        "batt": batt.astype(bf),
        "wk": np.ascontiguousarray(Wk.T.reshape(4, 128, H).astype(bf)),
        "wq": np.ascontiguousarray(Wq.T.reshape(4, 128, H).astype(bf)),
        "v": np.ascontiguousarray(v.reshape(2, 128).T.astype(np.float32)),
    }

    in_maps = []
    for c in range(NCORES):
        bs = slice(c * BL, (c + 1) * BL)
        # unified x [384rows, 128t, 12cols]: ctx cols 0:2 (fwd order, 128t),
        # opt cols 2:12 duplicated at t 0:64 and 64:128
        xrows = np.zeros((384, LC, NBM), np.float32)
        xc = ctx[bs]                                  # [2, 128, 300]
        xo = opts[bs].reshape(NI, LO, E)              # [10, 64, 300]
        xrows[:E, :, 0:BL] = xc.transpose(2, 1, 0)
        xrows[E, :, 0:BL] = 1.0
        xrows[:E, 0:LO, BL:] = xo.transpose(2, 1, 0)
        xrows[:E, LO:, BL:] = xo.transpose(2, 1, 0)
        xrows[E, :, BL:] = 1.0
        m = dict(shared)
        m["xu"] = np.ascontiguousarray(
            xrows.reshape(3, 128, LC, NBM).reshape(3, 128, LC * NBM).astype(bf))
        in_maps.append(m)
    return in_maps


def kernel(**inputs):
    if "nc" not in _CACHE:
        _CACHE["nc"] = _build()
    nc = _CACHE["nc"]
    in_maps = _prep_inputs(inputs)
    res = bass_utils.run_bass_kernel_spmd(nc, in_maps,
                                          core_ids=list(range(NCORES)))
    _CACHE["last_exec_ns"] = res.exec_time_ns
    logits = np.concatenate(
        [np.asarray(res.results[c]["out"], np.float32).reshape(BL, NOPT)
         for c in range(NCORES)], axis=0)
    x = logits - logits.max(axis=1, keepdims=True)
    ex = np.exp(x)
    return (ex / ex.sum(axis=1, keepdims=True)).astype(np.float32)


if __name__ == "__main__":
    _build()
    print("build+compile OK")
